# revision 6
# baseline (speedup 1.0000x reference)
"""CLAM instance-attention head on 8 Trainium2 NeuronCores (Bass/Tile).

Strategy
--------
Shard the instance dim N=50000 across 8 cores (6250 each, padded to 6272 =
14 blocks x 448). Everything on-device runs in "T-form": features on SBUF
partitions, instances on the free dim, so every GEMM is a native
lhsT.T @ rhs with no on-chip transposes. The host pre-transposes/pre-packs
h once (numpy) into the exact SBUF layout so each 448-instance block loads
with a single contiguous DMA (DMA-issue slots on an engine cost ~0.6us
each and serialize; with 8 DMAs/block the issue stream alone stalled the
PE ~24us at startup).

Per core the device computes, blockwise (448 instances at a time):
    hhT   = relu(fc_WT.T @ xT + fc_b)          [512 x n]   (fp32r matmuls)
    aT    = tanh(att_a_WT.T @ hhT + a_b)       [256 x n]
    bT    = sigmoid(att_b_WT.T @ hhT + b_b)    [256 x n]
    A_rawT= att_c_WT.T @ (aT*bT) + c_b         [1 x n]
    clsT  = cls_WT.T @ hhT + cls_b             [2 x n]
and writes A_rawT [1,6272] and clsT [2,6272] back to DRAM. A short run of
zero "warm-up" matmuls at kernel start keeps the PE busy through the HAM
activity window so real matmuls run at 2.4 GHz from the first block.

fp32r matmuls (TF32-like, ~1 cycle/row at free-dim >= 256) measured ~9e-4
max rel err on a K=1024 GEMM - 15x better than bf16 at identical speed,
which keeps the global top-k indices stable vs the fp32 reference.

The tiny cross-core reductions are exact fp64 host post-processing on the
gathered 50000-row outputs: softmax-weighted preds, stable top-8/bottom-8
of A_raw, and the 16-instance loss (hh rows recomputed on host in fp64
from the original fp32 inputs - 16x1024x512 FLOPs, microseconds).
"""

import sys

for _p in ("/opt/trn_rl_repo", "/root/.axon_site/_ro/trn_rl_repo"):
    if _p not in sys.path:
        sys.path.insert(0, _p)

import numpy as np

N, D_IN, D_H, D_A, N_CLS, K = 50000, 1024, 512, 256, 2, 8
NCORES = 8
NPC = 6250          # real instances per core
BLK = 448           # instances per block (>=256 keeps fp32r at 1 cycle/row)
NBLK = 14
NPAD = BLK * NBLK   # 6272 padded instances per core
KC = D_IN // 128    # 8 k-chunks over d_in
DHC = D_H // 128    # 4 chunks over d_h
DAC = D_A // 128    # 2 chunks over d_a
NWARM = 32          # PE warm-up matmuls at kernel start

_CACHE = {}


def _build_program():
    import concourse.bacc as bacc
    import concourse.mybir as mybir
    from concourse import tile

    f32 = mybir.dt.float32
    f32r = mybir.dt.float32r
    AF = mybir.ActivationFunctionType

    nc = bacc.Bacc(None)

    # All inputs arrive host-packed in SBUF layout (partition-major),
    # one contiguous DMA each.
    xT = nc.dram_tensor("xT", [NBLK, 128, KC * BLK], f32, kind="ExternalInput")
    fcw_d = nc.dram_tensor("fcw", [128, KC * D_H], f32, kind="ExternalInput")
    aw_d = nc.dram_tensor("aw", [128, DHC * D_A], f32, kind="ExternalInput")
    bw_d = nc.dram_tensor("bw", [128, DHC * D_A], f32, kind="ExternalInput")
    cw_d = nc.dram_tensor("cw", [128, DAC], f32, kind="ExternalInput")
    clsw_d = nc.dram_tensor("clsw", [128, DHC * N_CLS], f32, kind="ExternalInput")
    fcb_d = nc.dram_tensor("fcb", [128, DHC], f32, kind="ExternalInput")
    ab_d = nc.dram_tensor("ab", [128, DAC], f32, kind="ExternalInput")
    bb_d = nc.dram_tensor("bb", [128, DAC], f32, kind="ExternalInput")
    cb_d = nc.dram_tensor("cb", [1, 1], f32, kind="ExternalInput")
    clsb_d = nc.dram_tensor("clsb", [N_CLS, 1], f32, kind="ExternalInput")

    araw_out = nc.dram_tensor("araw", [1, NPAD], f32, kind="ExternalOutput")
    cls_out = nc.dram_tensor("clsout", [N_CLS, NPAD], f32, kind="ExternalOutput")

    with tile.TileContext(nc) as tc:
        with (
            tc.tile_pool(name="wpool", bufs=1) as wpool,
            tc.tile_pool(name="xpool", bufs=3) as xpool,
            tc.tile_pool(name="hpool", bufs=2 * DHC) as hpool,
            tc.tile_pool(name="abpool", bufs=2 * DAC) as abpool,
            tc.tile_pool(name="rowpool", bufs=1) as rowpool,
            tc.tile_pool(name="fc_ps", bufs=2, space="PSUM") as fc_ps,
            tc.tile_pool(name="att_ps", bufs=4, space="PSUM") as att_ps,
            tc.tile_pool(name="sm_ps", bufs=1, space="PSUM") as sm_ps,
        ):
            # --- PE warm-up: zero matmuls to lift the HAM clock gate ---
            warm = wpool.tile([128, BLK], mybir.dt.bfloat16, tag="warm")
            nc.gpsimd.memset(warm[:], 0.0)
            for i in range(NWARM):
                tag = "psA" if i % 2 == 0 else "pscls"
                ps = sm_ps.tile([1, BLK], f32, tag=tag)
                nc.tensor.matmul(ps[:], warm[:, 0:1], warm[:], start=True, stop=True)

            # --- resident weights (fp32r via bitcast DMA) + biases ---
            # weights issue on vector, x blocks alternate sync/gpsimd so no
            # single engine serializes the DMA descriptor pushes.
            fcw = wpool.tile([128, KC * D_H], f32r, tag="fcw")
            nc.scalar.dma_start(fcw[:], fcw_d[:, :].bitcast(f32r))
            aw = wpool.tile([128, DHC * D_A], f32r, tag="aw")
            nc.scalar.dma_start(aw[:], aw_d[:, :].bitcast(f32r))
            bw = wpool.tile([128, DHC * D_A], f32r, tag="bw")
            nc.scalar.dma_start(bw[:], bw_d[:, :].bitcast(f32r))
            cw = wpool.tile([128, DAC], f32r, tag="cw")
            nc.scalar.dma_start(cw[:], cw_d[:, :].bitcast(f32r))
            clsw = wpool.tile([128, DHC * N_CLS], f32r, tag="clsw")
            nc.scalar.dma_start(clsw[:], clsw_d[:, :].bitcast(f32r))
            fcb = wpool.tile([128, DHC], f32, tag="fcb")
            nc.scalar.dma_start(fcb[:], fcb_d[:, :])
            abias = wpool.tile([128, DAC], f32, tag="abias")
            nc.scalar.dma_start(abias[:], ab_d[:, :])
            bbias = wpool.tile([128, DAC], f32, tag="bbias")
            nc.scalar.dma_start(bbias[:], bb_d[:, :])
            cbias = wpool.tile([1, 1], f32, tag="cbias")
            nc.scalar.dma_start(cbias[:], cb_d[:, :])
            clsb = wpool.tile([N_CLS, 1], f32, tag="clsb")
            nc.scalar.dma_start(clsb[:], clsb_d[:, :])

            araw_sb = rowpool.tile([1, NPAD], f32, tag="araw_sb")
            cls_sb = rowpool.tile([N_CLS, NPAD], f32, tag="cls_sb")

            for b in range(NBLK):
                xt = xpool.tile([128, KC * BLK], f32r, tag="xt")
                dma_eng = nc.sync if b % 2 == 0 else nc.gpsimd
                dma_eng.dma_start(xt[:], xT[b].bitcast(f32r))

                # hhT = relu(fc_WT.T @ xT + fc_b), kept fp32r for reuse
                hh = []
                for dh in range(DHC):
                    ps = fc_ps.tile([128, BLK], f32, tag="fc")
                    for kc in range(KC):
                        nc.tensor.matmul(
                            ps[:],
                            fcw[:, kc * D_H + dh * 128:kc * D_H + (dh + 1) * 128],
                            xt[:, kc * BLK:(kc + 1) * BLK],
                            start=(kc == 0), stop=(kc == KC - 1),
                        )
                    h = hpool.tile([128, BLK], f32r, tag="hh")
                    nc.scalar.activation(h[:], ps[:], AF.Relu, bias=fcb[:, dh:dh + 1])
                    hh.append(h)

                # gated attention: ab = tanh(.) * sigmoid(.)
                ab = []
                for da in range(DAC):
                    psa = att_ps.tile([128, BLK], f32, tag="att")
                    for dh in range(DHC):
                        nc.tensor.matmul(
                            psa[:],
                            aw[:, dh * D_A + da * 128:dh * D_A + (da + 1) * 128],
                            hh[dh][:],
                            start=(dh == 0), stop=(dh == DHC - 1),
                        )
                    psb = att_ps.tile([128, BLK], f32, tag="att")
                    for dh in range(DHC):
                        nc.tensor.matmul(
                            psb[:],
                            bw[:, dh * D_A + da * 128:dh * D_A + (da + 1) * 128],
                            hh[dh][:],
                            start=(dh == 0), stop=(dh == DHC - 1),
                        )
                    at = abpool.tile([128, BLK], f32, tag="at")
                    nc.scalar.activation(at[:], psa[:], AF.Tanh, bias=abias[:, da:da + 1])
                    bt = abpool.tile([128, BLK], f32, tag="bt")
                    nc.scalar.activation(bt[:], psb[:], AF.Sigmoid, bias=bbias[:, da:da + 1])
                    abt = abpool.tile([128, BLK], f32r, tag="abt")
                    nc.vector.tensor_mul(abt[:], at[:], bt[:])
                    ab.append(abt)

                psA = sm_ps.tile([1, BLK], f32, tag="psA")
                for da in range(DAC):
                    nc.tensor.matmul(psA[:], cw[:, da:da + 1], ab[da][:],
                                     start=(da == 0), stop=(da == DAC - 1))
                nc.scalar.activation(araw_sb[0:1, b * BLK:(b + 1) * BLK], psA[:],
                                     AF.Identity, bias=cbias[:, 0:1])

                pscls = sm_ps.tile([N_CLS, BLK], f32, tag="pscls")
                for dh in range(DHC):
                    nc.tensor.matmul(pscls[:],
                                     clsw[:, dh * N_CLS:(dh + 1) * N_CLS],
                                     hh[dh][:],
                                     start=(dh == 0), stop=(dh == DHC - 1))
                nc.scalar.activation(cls_sb[0:N_CLS, b * BLK:(b + 1) * BLK], pscls[:],
                                     AF.Identity, bias=clsb[:, 0:1])

            nc.scalar.dma_start(araw_out[:, :], araw_sb[:])
            nc.scalar.dma_start(cls_out[:, :], cls_sb[:])

    nc.finalize()
    return nc


def _prep_inputs(h, fc_W, att_a_W, att_b_W, att_c_W, cls_W,
                 fc_b, att_a_b, att_b_b, att_c_b, cls_b):
    x = np.ascontiguousarray(h.reshape(N, D_IN), dtype=np.float32)

    def pack_w(wt, chunks):
        # [chunks*128, F] feature-major -> [128, chunks*F] partition-major
        f = wt.shape[1]
        return np.ascontiguousarray(
            wt.reshape(chunks, 128, f).transpose(1, 0, 2).reshape(128, chunks * f)
        )

    shared = {
        "fcw": pack_w(np.ascontiguousarray(fc_W.T), KC),
        "aw": pack_w(np.ascontiguousarray(att_a_W.T), DHC),
        "bw": pack_w(np.ascontiguousarray(att_b_W.T), DHC),
        "cw": pack_w(np.ascontiguousarray(att_c_W.T), DAC),
        "clsw": pack_w(np.ascontiguousarray(cls_W.T), DHC),
        "fcb": np.ascontiguousarray(fc_b.reshape(DHC, 128).T),
        "ab": np.ascontiguousarray(att_a_b.reshape(DAC, 128).T),
        "bb": np.ascontiguousarray(att_b_b.reshape(DAC, 128).T),
        "cb": np.ascontiguousarray(att_c_b.reshape(1, 1)),
        "clsb": np.ascontiguousarray(cls_b.reshape(N_CLS, 1)),
    }
    for k, v in shared.items():
        shared[k] = v.astype(np.float32, copy=False)
    in_maps = []
    for c in range(NCORES):
        xc = np.zeros((D_IN, NPAD), dtype=np.float32)
        xc[:, :NPC] = x[c * NPC:(c + 1) * NPC].T
        # [1024, 6272] -> per block [128, KC*BLK] partition-major
        xt = np.ascontiguousarray(
            xc.reshape(KC, 128, NBLK, BLK).transpose(2, 1, 0, 3)
            .reshape(NBLK, 128, KC * BLK)
        )
        m = {"xT": xt}
        m.update(shared)
        in_maps.append(m)
    return in_maps


def run_device(inputs, trace=False, trace_kwargs=None):
    """Build/compile (cached), run on 8 cores, return (results, BassKernelResults)."""
    from concourse.bass_utils import run_bass_kernel_spmd

    if "nc" not in _CACHE:
        _CACHE["nc"] = _build_program()
    nc = _CACHE["nc"]
    in_maps = _prep_inputs(
        inputs["h"], inputs["fc_W"], inputs["att_a_W"], inputs["att_b_W"],
        inputs["att_c_W"], inputs["cls_W"], inputs["fc_b"], inputs["att_a_b"],
        inputs["att_b_b"], inputs["att_c_b"], inputs["cls_b"],
    )
    kw = dict(trace_kwargs or {})
    br = run_bass_kernel_spmd(nc, in_maps, list(range(NCORES)), trace=trace, **kw)
    return br.results, br


def kernel(h, fc_W, fc_b, att_a_W, att_a_b, att_b_W, att_b_b,
           att_c_W, att_c_b, cls_W, cls_b, inst_W, inst_b, label):
    h = np.asarray(h, dtype=np.float32)
    inputs = dict(h=h, fc_W=np.asarray(fc_W, np.float32), fc_b=np.asarray(fc_b, np.float32),
                  att_a_W=np.asarray(att_a_W, np.float32), att_a_b=np.asarray(att_a_b, np.float32),
                  att_b_W=np.asarray(att_b_W, np.float32), att_b_b=np.asarray(att_b_b, np.float32),
                  att_c_W=np.asarray(att_c_W, np.float32), att_c_b=np.asarray(att_c_b, np.float32),
                  cls_W=np.asarray(cls_W, np.float32), cls_b=np.asarray(cls_b, np.float32))
    results, _ = run_device(inputs)

    araw = np.concatenate([results[c]["araw"][0, :NPC] for c in range(NCORES)])
    inst_sc = np.concatenate(
        [results[c]["clsout"][:, :NPC].T for c in range(NCORES)], axis=0
    )
    A_raw = araw[:, None].astype(np.float32)
    instance_scores = np.ascontiguousarray(inst_sc.astype(np.float32))

    # softmax-weighted bag prediction (exact, fp64)
    a64 = araw.astype(np.float64)
    e = np.exp(a64 - a64.max())
    preds = (e @ inst_sc.astype(np.float64)) / e.sum()
    preds = preds[None, :].astype(np.float32)

    # instance eval: global top-8 / bottom-8 of A_raw, loss from host-side
    # fp64 recompute of the 16 selected hh rows (exact vs fp32 reference)
    scores = araw
    top_p = np.argsort(-scores, kind="stable")[:K]
    top_n = np.argsort(scores, kind="stable")[:K]
    ids = np.concatenate([top_p, top_n])
    x16 = h.reshape(N, D_IN)[ids].astype(np.float64)
    hh16 = np.maximum(x16 @ np.asarray(fc_W, np.float64).T + np.asarray(fc_b, np.float64), 0.0)
    logits = hh16 @ np.asarray(inst_W, np.float64).T + np.asarray(inst_b, np.float64)
    m = logits.max(axis=1, keepdims=True)
    logp = logits - (m + np.log(np.exp(logits - m).sum(axis=1, keepdims=True)))
    targets = np.concatenate([np.ones(K, np.int64), np.zeros(K, np.int64)])
    instance_loss = np.float32(-logp[np.arange(2 * K), targets].mean())

    return preds, instance_scores, A_raw, instance_loss


# revision 8
# speedup vs baseline: 1.0244x; 1.0244x over previous
"""CLAM instance-attention head on 8 Trainium2 NeuronCores (Bass/Tile).

Strategy
--------
Shard the instance dim N=50000 across 8 cores (6250 each, padded to 6272 =
14 blocks x 448). Everything on-device runs in "T-form": features on SBUF
partitions, instances on the free dim, so every GEMM is a native
lhsT.T @ rhs with no on-chip transposes. The host pre-transposes/pre-packs
h once (numpy) into the exact SBUF layout so each 448-instance block loads
with a single contiguous DMA (DMA-issue slots on an engine cost ~0.6us
each and serialize; with 8 DMAs/block the issue stream alone stalled the
PE ~24us at startup).

Per core the device computes, blockwise (448 instances at a time):
    hhT   = relu(fc_WT.T @ xT + fc_b)          [512 x n]   (fp32r matmuls)
    aT    = tanh(att_a_WT.T @ hhT + a_b)       [256 x n]
    bT    = sigmoid(att_b_WT.T @ hhT + b_b)    [256 x n]
    A_rawT= att_c_WT.T @ (aT*bT) + c_b         [1 x n]
    clsT  = cls_WT.T @ hhT + cls_b             [2 x n]
and writes A_rawT [1,6272] and clsT [2,6272] back to DRAM. A short run of
zero "warm-up" matmuls at kernel start keeps the PE busy through the HAM
activity window so real matmuls run at 2.4 GHz from the first block.

fp32r matmuls (TF32-like, ~1 cycle/row at free-dim >= 256) measured ~9e-4
max rel err on a K=1024 GEMM - 15x better than bf16 at identical speed,
which keeps the global top-k indices stable vs the fp32 reference.

The tiny cross-core reductions are exact fp64 host post-processing on the
gathered 50000-row outputs: softmax-weighted preds, stable top-8/bottom-8
of A_raw, and the 16-instance loss (hh rows recomputed on host in fp64
from the original fp32 inputs - 16x1024x512 FLOPs, microseconds).
"""

import sys

for _p in ("/opt/trn_rl_repo", "/root/.axon_site/_ro/trn_rl_repo"):
    if _p not in sys.path:
        sys.path.insert(0, _p)

import numpy as np

N, D_IN, D_H, D_A, N_CLS, K = 50000, 1024, 512, 256, 2, 8
NCORES = 8
NPC = 6250          # real instances per core
BLK = 448           # instances per block (>=256 keeps fp32r at 1 cycle/row)
NBLK = 14
NPAD = BLK * NBLK   # 6272 padded instances per core
KC = D_IN // 128    # 8 k-chunks over d_in
DHC = D_H // 128    # 4 chunks over d_h
DAC = D_A // 128    # 2 chunks over d_a
NWARM = 40          # PE warm-up matmuls at kernel start

_CACHE = {}


def _build_program():
    import concourse.bacc as bacc
    import concourse.mybir as mybir
    from concourse import tile

    f32 = mybir.dt.float32
    f32r = mybir.dt.float32r
    AF = mybir.ActivationFunctionType

    nc = bacc.Bacc(None)

    # All inputs arrive host-packed in SBUF layout (partition-major),
    # one contiguous DMA each.
    xT = nc.dram_tensor("xT", [NBLK, 128, KC * BLK], f32, kind="ExternalInput")
    fcw_d = nc.dram_tensor("fcw", [128, KC * D_H], f32, kind="ExternalInput")
    aw_d = nc.dram_tensor("aw", [128, DHC * D_A], f32, kind="ExternalInput")
    bw_d = nc.dram_tensor("bw", [128, DHC * D_A], f32, kind="ExternalInput")
    cw_d = nc.dram_tensor("cw", [128, DAC], f32, kind="ExternalInput")
    clsw_d = nc.dram_tensor("clsw", [128, DHC * N_CLS], f32, kind="ExternalInput")
    fcb_d = nc.dram_tensor("fcb", [128, DHC], f32, kind="ExternalInput")
    ab_d = nc.dram_tensor("ab", [128, DAC], f32, kind="ExternalInput")
    bb_d = nc.dram_tensor("bb", [128, DAC], f32, kind="ExternalInput")
    cb_d = nc.dram_tensor("cb", [1, 1], f32, kind="ExternalInput")
    clsb_d = nc.dram_tensor("clsb", [N_CLS, 1], f32, kind="ExternalInput")

    araw_out = nc.dram_tensor("araw", [1, NPAD], f32, kind="ExternalOutput")
    cls_out = nc.dram_tensor("clsout", [N_CLS, NPAD], f32, kind="ExternalOutput")

    with tile.TileContext(nc) as tc:
        with (
            tc.tile_pool(name="wpool", bufs=1) as wpool,
            tc.tile_pool(name="xpool", bufs=3) as xpool,
            tc.tile_pool(name="hpool", bufs=2 * DHC) as hpool,
            tc.tile_pool(name="abpool", bufs=2 * DAC) as abpool,
            tc.tile_pool(name="rowpool", bufs=1) as rowpool,
            tc.tile_pool(name="fc_ps", bufs=2, space="PSUM") as fc_ps,
            tc.tile_pool(name="att_ps", bufs=4, space="PSUM") as att_ps,
            tc.tile_pool(name="sm_ps", bufs=1, space="PSUM") as sm_ps,
        ):
            # --- PE warm-up: zero matmuls to lift the HAM clock gate ---
            warm = wpool.tile([128, BLK], mybir.dt.bfloat16, tag="warm")
            nc.vector.memset(warm[:], 0.0)
            for i in range(NWARM):
                tag = "psA" if i % 2 == 0 else "pscls"
                ps = sm_ps.tile([1, BLK], f32, tag=tag)
                nc.tensor.matmul(ps[:], warm[:, 0:1], warm[:], start=True, stop=True)

            # --- resident weights (fp32r via bitcast DMA) + biases ---
            # A single DMA lands on a single HW queue (~100 GB/s), so the
            # 2MB fc weights and each 1.8MB x block are split across
            # several DMAs/queues. Issue streams are spread over
            # gpsimd (fc weights), sync (x blocks), scalar (small weights)
            # so no engine's ~0.6us-per-DMA issue cost serializes startup.
            fcw = wpool.tile([128, KC * D_H], f32r, tag="fcw")
            for q in range(4):
                sl = slice(q * KC * D_H // 4, (q + 1) * KC * D_H // 4)
                nc.gpsimd.dma_start(fcw[:, sl], fcw_d[:, sl].bitcast(f32r))
            fcb = wpool.tile([128, DHC], f32, tag="fcb")
            nc.scalar.dma_start(fcb[:], fcb_d[:, :])
            aw = wpool.tile([128, DHC * D_A], f32r, tag="aw")
            nc.scalar.dma_start(aw[:], aw_d[:, :].bitcast(f32r))
            bw = wpool.tile([128, DHC * D_A], f32r, tag="bw")
            nc.scalar.dma_start(bw[:], bw_d[:, :].bitcast(f32r))
            abias = wpool.tile([128, DAC], f32, tag="abias")
            nc.scalar.dma_start(abias[:], ab_d[:, :])
            bbias = wpool.tile([128, DAC], f32, tag="bbias")
            nc.scalar.dma_start(bbias[:], bb_d[:, :])
            cw = wpool.tile([128, DAC], f32r, tag="cw")
            nc.scalar.dma_start(cw[:], cw_d[:, :].bitcast(f32r))
            cbias = wpool.tile([1, 1], f32, tag="cbias")
            nc.scalar.dma_start(cbias[:], cb_d[:, :])
            clsw = wpool.tile([128, DHC * N_CLS], f32r, tag="clsw")
            nc.scalar.dma_start(clsw[:], clsw_d[:, :].bitcast(f32r))
            clsb = wpool.tile([N_CLS, 1], f32, tag="clsb")
            nc.scalar.dma_start(clsb[:], clsb_d[:, :])

            araw_sb = rowpool.tile([1, NPAD], f32, tag="araw_sb")
            cls_sb = rowpool.tile([N_CLS, NPAD], f32, tag="cls_sb")

            for b in range(NBLK):
                xt = xpool.tile([128, KC * BLK], f32r, tag="xt")
                nsplit = 4 if b == 0 else 2
                for q in range(nsplit):
                    sl = slice(q * KC * BLK // nsplit, (q + 1) * KC * BLK // nsplit)
                    eng = nc.sync if q % 2 == 0 else nc.gpsimd
                    eng.dma_start(xt[:, sl], xT[b][:, sl].bitcast(f32r))

                # hhT = relu(fc_WT.T @ xT + fc_b), kept fp32r for reuse
                hh = []
                for dh in range(DHC):
                    ps = fc_ps.tile([128, BLK], f32, tag="fc")
                    for kc in range(KC):
                        nc.tensor.matmul(
                            ps[:],
                            fcw[:, kc * D_H + dh * 128:kc * D_H + (dh + 1) * 128],
                            xt[:, kc * BLK:(kc + 1) * BLK],
                            start=(kc == 0), stop=(kc == KC - 1),
                        )
                    h = hpool.tile([128, BLK], f32r, tag="hh")
                    nc.scalar.activation(h[:], ps[:], AF.Relu, bias=fcb[:, dh:dh + 1])
                    hh.append(h)

                # gated attention: ab = tanh(.) * sigmoid(.)
                ab = []
                for da in range(DAC):
                    psa = att_ps.tile([128, BLK], f32, tag="att")
                    for dh in range(DHC):
                        nc.tensor.matmul(
                            psa[:],
                            aw[:, dh * D_A + da * 128:dh * D_A + (da + 1) * 128],
                            hh[dh][:],
                            start=(dh == 0), stop=(dh == DHC - 1),
                        )
                    psb = att_ps.tile([128, BLK], f32, tag="att")
                    for dh in range(DHC):
                        nc.tensor.matmul(
                            psb[:],
                            bw[:, dh * D_A + da * 128:dh * D_A + (da + 1) * 128],
                            hh[dh][:],
                            start=(dh == 0), stop=(dh == DHC - 1),
                        )
                    at = abpool.tile([128, BLK], f32, tag="at")
                    nc.scalar.activation(at[:], psa[:], AF.Tanh, bias=abias[:, da:da + 1])
                    bt = abpool.tile([128, BLK], f32, tag="bt")
                    nc.scalar.activation(bt[:], psb[:], AF.Sigmoid, bias=bbias[:, da:da + 1])
                    abt = abpool.tile([128, BLK], f32r, tag="abt")
                    nc.vector.tensor_mul(abt[:], at[:], bt[:])
                    ab.append(abt)

                psA = sm_ps.tile([1, BLK], f32, tag="psA")
                for da in range(DAC):
                    nc.tensor.matmul(psA[:], cw[:, da:da + 1], ab[da][:],
                                     start=(da == 0), stop=(da == DAC - 1))
                nc.scalar.activation(araw_sb[0:1, b * BLK:(b + 1) * BLK], psA[:],
                                     AF.Identity, bias=cbias[:, 0:1])

                pscls = sm_ps.tile([N_CLS, BLK], f32, tag="pscls")
                for dh in range(DHC):
                    nc.tensor.matmul(pscls[:],
                                     clsw[:, dh * N_CLS:(dh + 1) * N_CLS],
                                     hh[dh][:],
                                     start=(dh == 0), stop=(dh == DHC - 1))
                nc.scalar.activation(cls_sb[0:N_CLS, b * BLK:(b + 1) * BLK], pscls[:],
                                     AF.Identity, bias=clsb[:, 0:1])

            nc.scalar.dma_start(araw_out[:, :], araw_sb[:])
            nc.scalar.dma_start(cls_out[:, :], cls_sb[:])

    nc.finalize()
    return nc


def _prep_inputs(h, fc_W, att_a_W, att_b_W, att_c_W, cls_W,
                 fc_b, att_a_b, att_b_b, att_c_b, cls_b):
    x = np.ascontiguousarray(h.reshape(N, D_IN), dtype=np.float32)

    def pack_w(wt, chunks):
        # [chunks*128, F] feature-major -> [128, chunks*F] partition-major
        f = wt.shape[1]
        return np.ascontiguousarray(
            wt.reshape(chunks, 128, f).transpose(1, 0, 2).reshape(128, chunks * f)
        )

    shared = {
        "fcw": pack_w(np.ascontiguousarray(fc_W.T), KC),
        "aw": pack_w(np.ascontiguousarray(att_a_W.T), DHC),
        "bw": pack_w(np.ascontiguousarray(att_b_W.T), DHC),
        "cw": pack_w(np.ascontiguousarray(att_c_W.T), DAC),
        "clsw": pack_w(np.ascontiguousarray(cls_W.T), DHC),
        "fcb": np.ascontiguousarray(fc_b.reshape(DHC, 128).T),
        "ab": np.ascontiguousarray(att_a_b.reshape(DAC, 128).T),
        "bb": np.ascontiguousarray(att_b_b.reshape(DAC, 128).T),
        "cb": np.ascontiguousarray(att_c_b.reshape(1, 1)),
        "clsb": np.ascontiguousarray(cls_b.reshape(N_CLS, 1)),
    }
    for k, v in shared.items():
        shared[k] = v.astype(np.float32, copy=False)
    in_maps = []
    for c in range(NCORES):
        xc = np.zeros((D_IN, NPAD), dtype=np.float32)
        xc[:, :NPC] = x[c * NPC:(c + 1) * NPC].T
        # [1024, 6272] -> per block [128, KC*BLK] partition-major
        xt = np.ascontiguousarray(
            xc.reshape(KC, 128, NBLK, BLK).transpose(2, 1, 0, 3)
            .reshape(NBLK, 128, KC * BLK)
        )
        m = {"xT": xt}
        m.update(shared)
        in_maps.append(m)
    return in_maps


def run_device(inputs, trace=False, trace_kwargs=None):
    """Build/compile (cached), run on 8 cores, return (results, BassKernelResults)."""
    from concourse.bass_utils import run_bass_kernel_spmd

    if "nc" not in _CACHE:
        _CACHE["nc"] = _build_program()
    nc = _CACHE["nc"]
    in_maps = _prep_inputs(
        inputs["h"], inputs["fc_W"], inputs["att_a_W"], inputs["att_b_W"],
        inputs["att_c_W"], inputs["cls_W"], inputs["fc_b"], inputs["att_a_b"],
        inputs["att_b_b"], inputs["att_c_b"], inputs["cls_b"],
    )
    kw = dict(trace_kwargs or {})
    br = run_bass_kernel_spmd(nc, in_maps, list(range(NCORES)), trace=trace, **kw)
    return br.results, br


def kernel(h, fc_W, fc_b, att_a_W, att_a_b, att_b_W, att_b_b,
           att_c_W, att_c_b, cls_W, cls_b, inst_W, inst_b, label):
    h = np.asarray(h, dtype=np.float32)
    inputs = dict(h=h, fc_W=np.asarray(fc_W, np.float32), fc_b=np.asarray(fc_b, np.float32),
                  att_a_W=np.asarray(att_a_W, np.float32), att_a_b=np.asarray(att_a_b, np.float32),
                  att_b_W=np.asarray(att_b_W, np.float32), att_b_b=np.asarray(att_b_b, np.float32),
                  att_c_W=np.asarray(att_c_W, np.float32), att_c_b=np.asarray(att_c_b, np.float32),
                  cls_W=np.asarray(cls_W, np.float32), cls_b=np.asarray(cls_b, np.float32))
    results, _ = run_device(inputs)

    araw = np.concatenate([results[c]["araw"][0, :NPC] for c in range(NCORES)])
    inst_sc = np.concatenate(
        [results[c]["clsout"][:, :NPC].T for c in range(NCORES)], axis=0
    )
    A_raw = araw[:, None].astype(np.float32)
    instance_scores = np.ascontiguousarray(inst_sc.astype(np.float32))

    # softmax-weighted bag prediction (exact, fp64)
    a64 = araw.astype(np.float64)
    e = np.exp(a64 - a64.max())
    preds = (e @ inst_sc.astype(np.float64)) / e.sum()
    preds = preds[None, :].astype(np.float32)

    # instance eval: global top-8 / bottom-8 of A_raw, loss from host-side
    # fp64 recompute of the 16 selected hh rows (exact vs fp32 reference)
    scores = araw
    top_p = np.argsort(-scores, kind="stable")[:K]
    top_n = np.argsort(scores, kind="stable")[:K]
    ids = np.concatenate([top_p, top_n])
    x16 = h.reshape(N, D_IN)[ids].astype(np.float64)
    hh16 = np.maximum(x16 @ np.asarray(fc_W, np.float64).T + np.asarray(fc_b, np.float64), 0.0)
    logits = hh16 @ np.asarray(inst_W, np.float64).T + np.asarray(inst_b, np.float64)
    m = logits.max(axis=1, keepdims=True)
    logp = logits - (m + np.log(np.exp(logits - m).sum(axis=1, keepdims=True)))
    targets = np.concatenate([np.ones(K, np.int64), np.zeros(K, np.int64)])
    instance_loss = np.float32(-logp[np.arange(2 * K), targets].mean())

    return preds, instance_scores, A_raw, instance_loss
